# revision 48
# baseline (speedup 1.0000x reference)
"""Trainium2 Bass kernel for nn_BasicTransformerBlock (self-contained).

Sharding: sequence-parallel. 8 cores = 2 batch groups x 4 cores.
Each core owns TOK=512 tokens of one batch element. Attention needs the
full 2048-token context of that batch element, obtained by AllGathering
K^T and V(+ones column) within the 4-core group.

v4 (local-first attention):
  - Scores/exp for the LOCAL 512-token chunk run straight off kT_loc /
    vstage right after the Q projection, while the K/V AllGathers are in
    flight. Remote chunks are read back with per-core indirect DMAs
    (offset input `rbo`) that skip the core's own block, so the exp
    stream starts ~60us earlier and bridges the collective latency.
  - Projections run K -> Q -> V so local scores have Q as early as
    possible.
  - O-projection is folded into the head loop (per-head psum partials,
    DVE-added into xres) so the layer tail is just LN + K proj.
  - FF mixed precision: a-half bf16 (no x-quantization noise), gate
    half fp8 DoubleRow, FF2 fp8 DoubleRow — keeps rel err ~1.7e-2
    with most of the fp8 speedup.
  - fp8 weights pre-scaled out of the e4m3 denormal zone (wq x32,
    wk x16, wv x16, ones column 16, wf1-gate/wf2 x64); descales ride
    the exp / gelu activations and one DVE pass before the FF2 residual
    add. exp also gets bias=ln(16) so P=16*p clears fp8 denormals.

Layout conventions per core:
  x residual stream: token-major [512, 1024] fp32, as 4 tiles [128, 1024].
  xlnT: [1024, 512] fp8 as 4 tiles [128, 2, 512] (DoubleRow pair planes);
     the FF LN also emits a bf16 copy as 8 tiles [128, 512].
  Q^T, K^T: feature-major fp8 [1024, tok] as 8 tiles [128, tok]
     (tile m = heads 2m, 2m+1).
  V: token-major fp8 [tok, 1056] with a 16.0 column appended per head
     (column h*66+64), so the AV matmul also produces the softmax
     denominator (row 64 of the [66, tok] output).
  Scores S^T: [j, q] per head in [128, 2, 512] psum tiles (2 j-tiles);
     one Exp per psum tile -> P^T fp8 in SBUF with j-tile pairs in the
     middle dim, which is exactly the DoubleRow pair layout for AV.
  attn out^T: feature-major [1024, tok] bf16 after per-head 1/denom scale.
"""

import math
from contextlib import ExitStack

import numpy as np

import concourse.bass as bass
import concourse.mybir as mybir
import concourse.tile as tile
from concourse.tile_rust import add_dep_helper
from concourse import bacc
from concourse.masks import make_identity

F32 = mybir.dt.float32
BF16 = mybir.dt.bfloat16
F8 = mybir.dt.float8e4
I32 = mybir.dt.int32
AX = mybir.AxisListType.X
AF = mybir.ActivationFunctionType
ALU = mybir.AluOpType
DR = mybir.MatmulPerfMode.DoubleRow

D = 1024          # model dim
HEADS = 16
DH = 64
FF = 4096         # ff inner (per half)
EPS = 1e-5
P = 128

# fp8 pre-scales (all power-of-two; folded back out on-chip)
SQ = 32.0         # wq scale
SK = 16.0         # wk scale
SV = 16.0         # wv scale + the ones column value
SEXP = 1.0 / (SQ * SK)      # exp input descale (2^-9)
BEXP = math.log(16.0)       # exp bias: P = 16*p, clears fp8 denormals
SF = 64.0         # wf1-gate / wf2 scale
ISF = 1.0 / SF


class Ctx:
    """Holds the bass handles shared across emit stages."""


def build(group: int, tok: int, use_bias: bool = False):
    """group: cores per batch group (1 = no collective, 4 = real).
    tok: local tokens per core (512)."""
    ntok = group * tok
    TT = tok // P          # local token tiles (4)
    JT = ntok // P         # context token tiles (16 when group=4)

    nc = bacc.Bacc("TRN2", target_bir_lowering=False, debug=False,
                   num_devices=8)

    c = Ctx()
    c.nc = nc
    c.group = group
    c.tok = tok
    c.ntok = ntok
    c.TT = TT
    c.JT = JT
    c.use_bias = use_bias
    c.gate_ins = None      # prev layer's K gather: gates weight prefetch

    # ---- I/O ----
    c.x_in = nc.dram_tensor("x", [tok, D], F32, kind="ExternalInput")
    c.y_out = nc.dram_tensor("y", [tok, D], F32, kind="ExternalOutput")
    w = {}
    for i in (1, 2):
        # DoubleRow-packed fp8 projection weights: [j, k, (i, m)] where
        # input dim = 256*j + 128*i + k
        for nm in ("wq", "wk", "wv"):
            w[f"{nm}{i}"] = nc.dram_tensor(f"{nm}{i}", [4, P, 2 * D], F8,
                                           kind="ExternalInput")
        w[f"wo{i}"] = nc.dram_tensor(f"wo{i}", [D, D], BF16,
                                     kind="ExternalInput")
    w["wf1a"] = nc.dram_tensor("wf1a", [32, P, 1024], BF16,
                               kind="ExternalInput")
    w["wf1g"] = nc.dram_tensor("wf1g", [32, P, 1024], F8,
                               kind="ExternalInput")
    w["wf2"] = nc.dram_tensor("wf2", [16, P, 2 * D], F8,
                              kind="ExternalInput")
    if group > 1:
        c.rbo_in = nc.dram_tensor("rbo", [P, group - 1], I32,
                                  kind="ExternalInput")
    if use_bias:
        for i in (1, 2):
            for nmv in ("cq", "ck", "cv", "bo"):
                w[f"{nmv}{i}"] = nc.dram_tensor(f"{nmv}{i}", [1, D], BF16,
                                                kind="ExternalInput")
        w["c1"] = nc.dram_tensor("c1", [1, 2 * FF], BF16, kind="ExternalInput")
        w["bf2"] = nc.dram_tensor("bf2", [1, D], BF16, kind="ExternalInput")
    c.w = w

    with ExitStack() as stack:
        tc = stack.enter_context(tile.TileContext(nc))
        c.tc = tc

        const = stack.enter_context(tc.tile_pool(name="const", bufs=1))
        c.identity = const.tile([P, P], BF16, name="identity")
        make_identity(nc, c.identity)
        c.ones_bf = const.tile([1, tok], BF16, name="ones_bf")
        nc.vector.memset(c.ones_bf, 1.0)
        c.eps_tile = const.tile([P, 1], F32, name="eps_tile")
        nc.vector.memset(c.eps_tile, EPS)
        c.bexp_tile = const.tile([P, 1], F32, name="bexp_tile")
        nc.vector.memset(c.bexp_tile, BEXP)
        if group > 1:
            c.rbo_sb = const.tile([P, group - 1], I32, name="rbo_sb")
            nc.sync.dma_start(c.rbo_sb, c.rbo_in[:, :])
        if use_bias:
            c.bias_sb = {}
            for key, t in w.items():
                if key[0] in "cb" and key not in ("c1",):
                    bt = const.tile([1, D], BF16, name=f"sb_{key}")
                    nc.sync.dma_start(bt, t[:, :])
                    c.bias_sb[key] = bt
            bt = const.tile([1, 2 * FF], BF16, name="sb_c1")
            nc.sync.dma_start(bt, w["c1"][:, :])
            c.bias_sb["c1"] = bt

        xres_pool = stack.enter_context(tc.tile_pool(name="xres", bufs=1))
        c.xres = [xres_pool.tile([P, D], F32, name=f"xres{t}") for t in range(TT)]
        for t in range(TT):
            nc.sync.dma_start(c.xres[t], c.x_in[t * P:(t + 1) * P, :])

        # DRAM bounce buffers, partition-major: (p, m, f) / (p, t, f) so
        # staging and readback DMAs are 4KB+ contiguous per partition.
        if group > 1:
            kh = 8 * P * tok              # whole K^T (8 m-tiles), fp8
            vh = tok * HEADS * 66         # whole V(+ones+pad), fp8
            c.kh, c.vh = kh, vh
            dram = stack.enter_context(
                tc.tile_pool(name="dram", bufs=1, space="DRAM"))
            c.k_in = [dram.tile([kh], F8, name=f"k_in{i}") for i in (0, 1)]
            c.k_out = [dram.tile([group * kh], F8, name=f"k_out{i}")
                       for i in (0, 1)]
            c.v_in = [dram.tile([vh], F8, name=f"v_in{i}") for i in (0, 1)]
            c.v_out = [dram.tile([group * vh], F8, name=f"v_out{i}")
                       for i in (0, 1)]
        emit_attn(c, 1)
        emit_attn(c, 2)
        emit_ff(c)

        for t in range(TT):
            nc.sync.dma_start(c.y_out[t * P:(t + 1) * P, :], c.xres[t])

    nc.compile()
    return nc


def emit_ln_transpose(c, outer, name, also_bf16=False):
    """LayerNorm xres (gain/bias pre-folded into weights) and produce
    xlnT (transposed normalized x) as 4 tiles [128, 2, tok] fp8 with
    DoubleRow pair planes. also_bf16: additionally emit 8 bf16 tiles
    [128, tok] (FF a-half path). Tiles live in `outer`."""
    nc, tc = c.nc, c.tc
    TT = c.TT

    xlnT_pool = outer.enter_context(tc.tile_pool(name=f"{name}_xlnT", bufs=1))
    xlnT = [xlnT_pool.tile([P, 2, c.tok], F8, name=f"{name}_xlnT{j}")
            for j in range(4)]
    xlnT_b = None
    if also_bf16:
        xlnT_b = [xlnT_pool.tile([P, c.tok], BF16, name=f"{name}_xlnTb{j}")
                  for j in range(8)]

    with ExitStack() as ph:
        pool = ph.enter_context(tc.tile_pool(name=f"{name}_ln", bufs=2))
        psum_sq = ph.enter_context(
            tc.tile_pool(name=f"{name}_psq", bufs=2, space="PSUM"))
        psum_tr = ph.enter_context(
            tc.tile_pool(name=f"{name}_ptr", bufs=4, space="PSUM"))

        for t in range(TT):
            xr = c.xres[t]
            # var = E[x^2] - mu^2: sum and sumsq run concurrently (DVE / ACT)
            ssum = pool.tile([P, 1], F32, tag="ssum", name=f"{name}_ssum{t}")
            nc.vector.reduce_sum(ssum, xr, axis=AX)
            sq_sink = psum_sq.tile([P, D], F32, tag="sq", name=f"{name}_sq{t}")
            sumsq = pool.tile([P, 1], F32, tag="sumsq", name=f"{name}_vs{t}")
            nc.scalar.activation(sq_sink, xr, AF.Square, accum_out=sumsq)
            mu = pool.tile([P, 1], F32, tag="mu", name=f"{name}_mu{t}")
            nc.vector.tensor_scalar_mul(mu, ssum, 1.0 / D)
            musq = pool.tile([P, 1], F32, tag="musq", name=f"{name}_msq{t}")
            nc.vector.tensor_mul(musq, mu, mu)
            bvar = pool.tile([P, 1], F32, tag="bvar", name=f"{name}_bv{t}")
            nc.vector.tensor_scalar(bvar, musq, -1.0, EPS,
                                    op0=ALU.mult, op1=ALU.add)
            std = pool.tile([P, 1], F32, tag="std", name=f"{name}_std{t}")
            nc.scalar.activation(std, sumsq, AF.Sqrt, bias=bvar,
                                 scale=1.0 / D)
            rstd = pool.tile([P, 1], F32, tag="rstd", name=f"{name}_rstd{t}")
            nc.vector.reciprocal(rstd, std)
            nmr = pool.tile([P, 1], F32, tag="nmr", name=f"{name}_nmr{t}")
            nc.vector.tensor_mul(nmr, mu, rstd)
            nc.vector.tensor_scalar_mul(nmr, nmr, -1.0)
            xln = pool.tile([P, D], BF16, tag="xln", name=f"{name}_xln{t}")
            nc.vector.tensor_scalar(xln, xr, rstd, nmr,
                                    op0=ALU.mult, op1=ALU.add)
            for dc in range(8):
                tp = psum_tr.tile([P, P], BF16, tag="tp",
                                  name=f"{name}_tp{t}_{dc}")
                nc.tensor.transpose(tp, xln[:, dc * P:(dc + 1) * P],
                                    c.identity)
                nc.vector.tensor_copy(
                    xlnT[dc // 2][:, dc % 2, t * P:(t + 1) * P], tp)
                if also_bf16:
                    nc.vector.tensor_copy(
                        xlnT_b[dc][:, t * P:(t + 1) * P], tp)
    return xlnT, xlnT_b


def load_w_dr(c, pool, name, w_dram, dep=None):
    """Preload the 4 DoubleRow-packed fp8 weight tiles of one projection.
    dep: instruction the DMAs must follow (weight-traffic gating)."""
    nc = c.nc
    w_tiles = []
    for j in range(4):
        wt = pool.tile([P, 2, D], F8, name=f"{name}_w{j}")
        dma = nc.sync.dma_start(wt.rearrange("p a b -> p (a b)"),
                                w_dram[j, :, :])
        if dep is not None:
            add_dep_helper(dma.ins, dep.ins, sync=True,
                           reason="weight load after prev-layer gather")
        w_tiles.append(wt)
    return w_tiles


def proj_fm_dr(c, name, xlnT, w_tiles, out_tiles, bias_key=None,
               bufs=3, after_m=None):
    """Feature-major fp8 DoubleRow projection:
    out^T[m] [128, tok] fp8 = W-chunk.T @ xln. after_m(m) fires after
    head-pair m's output copy is emitted (local-score interleaving)."""
    nc, tc = c.nc, c.tc
    with ExitStack() as sub:
        psum = sub.enter_context(
            tc.tile_pool(name=f"{name}_ps", bufs=bufs, space="PSUM"))
        has_bias = c.use_bias and bias_key is not None
        for m in range(8):
            ps = psum.tile([P, c.tok], F32, tag="proj", name=f"{name}_ps{m}")
            for j in range(4):
                nc.tensor.matmul(ps, lhsT=w_tiles[j][:, :, m * P:(m + 1) * P],
                                 rhs=xlnT[j], start=(j == 0),
                                 stop=(j == 3 and not has_bias),
                                 perf_mode=DR)
            if has_bias:
                nc.tensor.matmul(
                    ps, lhsT=c.bias_sb[bias_key][0:1, m * P:(m + 1) * P],
                    rhs=c.ones_bf, start=False, stop=True,
                    skip_group_check=True)
            nc.vector.tensor_copy(out_tiles[m], ps)
            if after_m is not None:
                after_m(m)


def emit_attn(c, idx):
    nc, tc = c.nc, c.tc
    name = f"a{idx}"
    TT, JT, tok = c.TT, c.JT, c.tok
    RG = [[0, 1, 2, 3], [4, 5, 6, 7]]
    vw = HEADS * 66          # 1056 (64 dh + ones + pad; 66 keeps the
    # vones pair stride a multiple of 16, a DoubleRow LDWEIGHTS ISA rule)
    NR = JT // 2             # score rounds per head (2 j-tiles per round)
    NL = TT // 2             # local rounds (from kT_loc / vstage)
    NREM = c.group - 1       # remote blocks

    with ExitStack() as ph:
        # prefetch all projection weights first — the DMAs land during the
        # LayerNorm so no projection matmul ever waits on a weight tile.
        # Layer 2's loads are gated behind layer 1's K gather so they
        # don't eat the HBM bandwidth the layer-1 collectives need.
        wqkv_pool = ph.enter_context(tc.tile_pool(name=f"{name}_wqkv",
                                                  bufs=1))
        wk_tiles = load_w_dr(c, wqkv_pool, f"{name}_wk", c.w[f"wk{idx}"],
                             dep=c.gate_ins)
        wq_tiles = load_w_dr(c, wqkv_pool, f"{name}_wq", c.w[f"wq{idx}"],
                             dep=c.gate_ins)
        wv_tiles = load_w_dr(c, wqkv_pool, f"{name}_wv", c.w[f"wv{idx}"],
                             dep=c.gate_ins)

        xlnT, _ = emit_ln_transpose(c, ph, name)

        kf = ph.enter_context(tc.tile_pool(name=f"{name}_kf", bufs=1))
        kT_loc = kf.tile([P, 8, tok], F8, name=f"{name}_kTl")
        vstage = kf.tile([P, TT, vw], F8, name=f"{name}_vst")
        qT = [kf.tile([P, tok], F8, name=f"{name}_qT{m}") for m in range(8)]
        if c.group > 1:
            kT_rem = kf.tile([P, NREM, 8, tok], F8, name=f"{name}_kTr")
            v_rem = kf.tile([P, NREM, TT, vw], F8, name=f"{name}_vr")

        # --- K^T projection, then one whole-K gather ---
        proj_fm_dr(c, f"{name}_kproj", xlnT, wk_tiles,
                   [kT_loc[:, m, :] for m in range(8)],
                   bias_key=f"ck{idx}")
        if c.group > 1:
            nc.sync.dma_start(
                c.k_in[idx - 1][:].rearrange("(p q) -> p q", p=P),
                kT_loc.rearrange("p m f -> p (m f)"))
            nc.gpsimd.collective_compute(
                "AllGather", ALU.bypass, replica_groups=RG,
                ins=[c.k_in[idx - 1][:]], outs=[c.k_out[idx - 1][:]])

        # --- Q^T projection (before V so local scores start early) ---
        proj_fm_dr(c, f"{name}_qproj", xlnT, wq_tiles, qT,
                   bias_key=f"cq{idx}")

        # --- V(+ones) projection, token-major fp8 DoubleRow ---
        with ExitStack() as sub:
            psum = sub.enter_context(
                tc.tile_pool(name=f"{name}_vps", bufs=2, space="PSUM"))
            nc.vector.memset(
                vstage.rearrange("p t (h e) -> p t h e", e=66)[:, :, :, 64:66],
                SV)
            pss = {}
            for t in range(TT):
                for n in range(2):
                    pss[(t, n)] = psum.tile([P, 512], F32, tag=f"vp{n}",
                                            name=f"{name}_vps{t}_{n}")
                for j in range(4):
                    for n in range(2):
                        nc.tensor.matmul(
                            pss[(t, n)],
                            lhsT=xlnT[j][:, :, t * P:(t + 1) * P],
                            rhs=wv_tiles[j][:, :, n * 512:(n + 1) * 512],
                            start=(j == 0),
                            stop=(j == 3 and not c.use_bias),
                            perf_mode=DR)
                for n in range(2):
                    if c.use_bias:
                        nc.tensor.matmul(
                            pss[(t, n)], lhsT=c.ones_bf[0:1, 0:P],
                            rhs=c.bias_sb[f"cv{idx}"][0:1,
                                                      n * 512:(n + 1) * 512],
                            start=False, stop=True, skip_group_check=True)
                    dst = vstage[:, t, n * 528:(n + 1) * 528].rearrange(
                        "p (h e) -> p h e", e=66)[:, :, 0:64]
                    nc.vector.tensor_copy(
                        dst, pss[(t, n)].rearrange("p (h e) -> p h e", e=64))
            if c.group > 1:
                nc.sync.dma_start(
                    c.v_in[idx - 1][:].rearrange("(p q) -> p q", p=P),
                    vstage.rearrange("p t f -> p (t f)"))
                ag_v = nc.gpsimd.collective_compute(
                    "AllGather", ALU.bypass, replica_groups=RG,
                    ins=[c.v_in[idx - 1][:]], outs=[c.v_out[idx - 1][:]])
                c.gate_ins = ag_v
        # --- remote readbacks: per-core indirect DMAs skip own block ---
        if c.group > 1:
            k_src = c.k_out[idx - 1][:].rearrange(
                "(r q) -> r q", q=8 * tok)          # [(4*128), 4096]
            v_src = c.v_out[idx - 1][:].rearrange(
                "(r q) -> r q", q=TT * vw)          # [(4*128), 4224]
            for i in range(NREM):
                nc.gpsimd.indirect_dma_start(
                    out=kT_rem[:, i].rearrange("p m f -> p (m f)"),
                    out_offset=None,
                    in_=k_src,
                    in_offset=bass.IndirectOffsetOnAxis(
                        ap=c.rbo_sb[:, i:i + 1], axis=0))
            for i in range(NREM):
                nc.gpsimd.indirect_dma_start(
                    out=v_rem[:, i].rearrange("p t f -> p (t f)"),
                    out_offset=None,
                    in_=v_src,
                    in_offset=bass.IndirectOffsetOnAxis(
                        ap=c.rbo_sb[:, i:i + 1], axis=0))

        # prefetch out-projection weights while heads run
        wo_pool = ph.enter_context(tc.tile_pool(name=f"{name}_wop", bufs=1))
        wo_tiles = []
        for m in range(8):
            wt = wo_pool.tile([P, D], BF16, name=f"{name}_wo{m}")
            nc.sync.dma_start(wt, c.w[f"wo{idx}"][m * P:(m + 1) * P, :])
            wo_tiles.append(wt)

        # --- attention: local-chunk scores for ALL heads first, then
        # remote chunks + AV/epilogue/O-proj pipelined per head ---
        attnT_pool = ph.enter_context(tc.tile_pool(name=f"{name}_at", bufs=1))
        attnT = [attnT_pool.tile([P, tok], BF16, name=f"{name}_attnT{m}")
                 for m in range(8)]

        sub = ExitStack()
        psum_sc = sub.enter_context(
            tc.tile_pool(name=f"{name}_psc", bufs=2, space="PSUM"))
        psum_av = sub.enter_context(
            tc.tile_pool(name=f"{name}_pav", bufs=2, space="PSUM"))
        psum_oh = sub.enter_context(
            tc.tile_pool(name=f"{name}_poh", bufs=2, space="PSUM"))
        pT_pool = sub.enter_context(
            tc.tile_pool(name=f"{name}_pT", bufs=68))
        small = sub.enter_context(
            tc.tile_pool(name=f"{name}_small", bufs=2))

        pend_p = {m: [] for m in range(8)}

        def emit_rounds(m, r0, r1):
            """Score rounds [r0, r1) for head-pair m. j-tiles 2r, 2r+1;
            jt < TT come from kT_loc, the rest from kT_rem."""
            for r in range(r0, r1):
                ps2 = [psum_sc.tile([P, 2, tok], F32, tag="sc",
                                    name=f"{name}_sc{m}_{r}_{s}")
                       for s in range(2)]
                for u in range(2):
                    jt = 2 * r + u
                    for s in range(2):
                        po = s * 64
                        if jt < TT:
                            lhsT = kT_loc[po:po + 64, m,
                                          jt * P:(jt + 1) * P]
                        else:
                            jr = jt - TT
                            lhsT = kT_rem[po:po + 64, jr // TT, m,
                                          (jr % TT) * P:(jr % TT + 1) * P]
                        nc.tensor.matmul(ps2[s][:, u, :], lhsT=lhsT,
                                         rhs=qT[m][po:po + 64, :],
                                         start=True, stop=True)
                for s in range(2):
                    p_sb = pT_pool.tile([P, 2, tok], F8, tag="pT",
                                        name=f"{name}_p{m}_{r}_{s}")
                    nc.scalar.activation(p_sb, ps2[s], AF.Exp,
                                         bias=c.bexp_tile, scale=SEXP)
                    pend_p[m].append(p_sb)

        def finish_head(m):
            """AV + epilogue + per-head O-projection for head-pair m."""
            av_pair = [psum_av.tile([P, tok], F32, tag="av",
                                    name=f"{name}_av{m}_{s}")
                       for s in range(2)]
            for rr in range(NR):
                jt0 = 2 * rr
                for s in range(2):
                    h = 2 * m + s
                    if jt0 < TT:
                        lhsT = vstage[:, jt0:jt0 + 2, h * 66:(h + 1) * 66]
                    else:
                        jr = jt0 - TT
                        lhsT = v_rem[:, jr // TT, (jr % TT):(jr % TT) + 2,
                                     h * 66:(h + 1) * 66]
                    nc.tensor.matmul(
                        av_pair[s][0:66, :], lhsT=lhsT,
                        rhs=pend_p[m][2 * rr + s],
                        start=(rr == 0), stop=(rr == NR - 1),
                        perf_mode=DR)
            pend_p[m] = None
            at_t = attnT[m]
            for s in range(2):
                po2 = s * 64
                # NB: DVE reciprocal_approx reads garbage from PSUM on
                # real HW (sim models it fine) — stage through SBUF.
                den1 = small.tile([1, tok], F32, tag="den",
                                  name=f"{name}_den{m}_{s}")
                nc.vector.tensor_copy(den1, av_pair[s][64:65, :])
                rden_f = small.tile([1, tok], F32, tag="rdenf",
                                    name=f"{name}_rdf{m}_{s}")
                nc.vector.reciprocal_approx_fast(rden_f, den1)
                # broadcast 1/den across the 64 dh rows on the (idle)
                # GpSimd engine — keeps the PE queue off the epilogue
                rbc = small.tile([64, tok], F32, tag="rbc",
                                 name=f"{name}_rbc{m}_{s}")
                nc.gpsimd.partition_broadcast(rbc, rden_f, channels=64)
                nc.vector.tensor_tensor(at_t[po2:po2 + 64, :],
                                        av_pair[s][0:64, :], rbc,
                                        op=ALU.mult)
            # per-head O-projection partial, added straight into xres
            for t in range(TT):
                for n in range(2):
                    ps = psum_oh.tile([P, 512], F32, tag="oh",
                                      name=f"{name}_oh{m}_{t}_{n}")
                    nc.tensor.matmul(
                        ps, lhsT=at_t[:, t * P:(t + 1) * P],
                        rhs=wo_tiles[m][:, n * 512:(n + 1) * 512],
                        start=True, stop=True)
                    sl = slice(n * 512, (n + 1) * 512)
                    nc.vector.tensor_add(c.xres[t][:, sl],
                                         c.xres[t][:, sl], ps)

        # phase L: local rounds for every head (exp stream starts
        # while the gathers are still in flight)
        for m in range(8):
            emit_rounds(m, 0, NL)
        # phase R: remote rounds, AV pipelined AVLAG heads behind
        AVLAG = 2 if c.group > 1 else 0
        if c.group > 1:
            for m in range(8):
                emit_rounds(m, NL, NR)
                if m >= AVLAG:
                    finish_head(m - AVLAG)
            for m in range(8 - AVLAG, 8):
                finish_head(m)
        else:
            for m in range(8):
                finish_head(m)
        sub.close()


def emit_ff(c):
    nc, tc = c.nc, c.tc
    name = "ff"
    TT, tok = c.TT, c.tok

    with ExitStack() as ph:
        xlnT, xlnT_b = emit_ln_transpose(c, ph, name, also_bf16=True)

        h2_pool = ph.enter_context(tc.tile_pool(name=f"{name}_h2", bufs=1))
        h2T = [h2_pool.tile([P, 2, tok], F8, name=f"{name}_h2T{jj}")
               for jj in range(16)]

        # FF2 weights: all 16 fp8 pair tiles resident (gated on attn2)
        wf2_pool = ph.enter_context(tc.tile_pool(name=f"{name}_w2", bufs=1))
        w2p = []
        for jj in range(16):
            w2 = wf2_pool.tile([P, 2, D], F8, name=f"{name}_w2_{jj}")
            dma = nc.sync.dma_start(w2.rearrange("p a b -> p (a b)"),
                                    c.w["wf2"][jj, :, :])
            if c.gate_ins is not None and jj == 0:
                add_dep_helper(dma.ins, c.gate_ins.ins, sync=True,
                               reason="FF2 weights after attn2 gather")
            w2p.append(w2)

        with ExitStack() as sub:
            f1_pool = sub.enter_context(
                tc.tile_pool(name=f"{name}_f1", bufs=6))
            psum_ff = sub.enter_context(
                tc.tile_pool(name=f"{name}_pff", bufs=3, space="PSUM"))
            gl_pool = sub.enter_context(
                tc.tile_pool(name=f"{name}_gl", bufs=3))

            for pm in range(32):
                f1a = f1_pool.tile([P, 8, P], BF16, tag="f1a",
                                   name=f"{name}_f1a_{pm}")
                dma = nc.sync.dma_start(
                    f1a.rearrange("p a b -> p (a b)"), c.w["wf1a"][pm, :, :])
                if c.gate_ins is not None and pm == 0:
                    add_dep_helper(dma.ins, c.gate_ins.ins, sync=True,
                                   reason="FF1 weights after attn2 gather")
                f1g = f1_pool.tile([P, 4, 2, P], F8, tag="f1g",
                                   name=f"{name}_f1g_{pm}")
                nc.sync.dma_start(
                    f1g.rearrange("p a b c -> p (a b c)"),
                    c.w["wf1g"][pm, :, :])
                ps_a = psum_ff.tile([P, tok], F32, tag="ffa",
                                    name=f"{name}_fa{pm}")
                ps_g = psum_ff.tile([P, tok], F32, tag="ffg",
                                    name=f"{name}_fg{pm}")
                for kd in range(8):
                    nc.tensor.matmul(ps_a, lhsT=f1a[:, kd, :],
                                     rhs=xlnT_b[kd], start=(kd == 0),
                                     stop=(kd == 7 and not c.use_bias))
                for j in range(4):
                    nc.tensor.matmul(ps_g, lhsT=f1g[:, j, :, :],
                                     rhs=xlnT[j], start=(j == 0),
                                     stop=(j == 3 and not c.use_bias),
                                     perf_mode=DR)
                if c.use_bias:
                    nc.tensor.matmul(
                        ps_a,
                        lhsT=c.bias_sb["c1"][0:1, pm * P:(pm + 1) * P],
                        rhs=c.ones_bf, start=False, stop=True,
                        skip_group_check=True)
                    nc.tensor.matmul(
                        ps_g,
                        lhsT=c.bias_sb["c1"][0:1,
                                             FF + pm * P:FF + (pm + 1) * P],
                        rhs=c.ones_bf, start=False, stop=True,
                        skip_group_check=True)
                # gelu(gate): descale the SF-scaled psum inside the ACT
                gl = gl_pool.tile([P, tok], BF16, tag="gelu",
                                  name=f"{name}_gl{pm}")
                nc.scalar.activation(gl, ps_g, AF.Gelu, scale=ISF)
                nc.vector.tensor_tensor(h2T[pm // 2][:, pm % 2, :],
                                        ps_a, gl, op=ALU.mult)

        # FF2 + residual (fp8 DoubleRow, descale 1/SF before the add)
        with ExitStack() as sub:
            psum_o = sub.enter_context(
                tc.tile_pool(name=f"{name}_po2", bufs=1, space="PSUM"))
            tmp_pool = sub.enter_context(
                tc.tile_pool(name=f"{name}_tmp", bufs=2))
            ps_o = {}
            for t in range(TT):
                for n in range(2):
                    ps_o[(t, n)] = psum_o.tile([P, 512], F32, tag=f"o{t}_{n}",
                                               name=f"{name}_pso{t}_{n}")
            for t in range(TT):
                for jj in range(16):
                    for n in range(2):
                        nc.tensor.matmul(
                            ps_o[(t, n)],
                            lhsT=h2T[jj][:, :, t * P:(t + 1) * P],
                            rhs=w2p[jj][:, :, n * 512:(n + 1) * 512],
                            start=(jj == 0),
                            stop=(jj == 15 and not c.use_bias),
                            perf_mode=DR)
                for n in range(2):
                    if c.use_bias:
                        nc.tensor.matmul(
                            ps_o[(t, n)], lhsT=c.ones_bf[0:1, 0:P],
                            rhs=c.bias_sb["bf2"][0:1, n * 512:(n + 1) * 512],
                            start=False, stop=True)
                    sl = slice(n * 512, (n + 1) * 512)
                    tmp = tmp_pool.tile([P, 512], F32, tag="ffo",
                                        name=f"{name}_ffo{t}_{n}")
                    nc.vector.tensor_scalar_mul(tmp, ps_o[(t, n)], ISF)
                    nc.vector.tensor_add(c.xres[t][:, sl], c.xres[t][:, sl],
                                         tmp)


# ---------------- host-side helpers ----------------

def prep_weights(inp):
    """Fold LN gains + attention scale into packed fp8/bf16 weights.
    fp8 weights are pre-scaled (SQ/SK/SV/SF) out of the e4m3 denormal
    zone; the kernel descales in the exp / gelu / FF2-output stages."""
    import ml_dtypes
    f = np.float32
    f8 = ml_dtypes.float8_e4m3
    bf = ml_dtypes.bfloat16
    out = {}

    def pack_dr(wt):
        # wt [1024 in, N out] -> [4, 128, 2N] with in = 256j + 128i + k
        n = wt.shape[1]
        return np.ascontiguousarray(
            wt.reshape(4, 2, P, n).transpose(0, 2, 1, 3)
            .reshape(4, P, 2 * n).astype(f8))

    for i in (1, 2):
        g = np.asarray(inp[f"ln{i}_g"], f)
        out[f"wq{i}"] = pack_dr(g[:, None] * np.asarray(inp[f"w_q{i}"], f).T
                                * np.float32(DH ** -0.5) * np.float32(SQ))
        out[f"wk{i}"] = pack_dr(g[:, None] * np.asarray(inp[f"w_k{i}"], f).T
                                * np.float32(SK))
        out[f"wv{i}"] = pack_dr(g[:, None] * np.asarray(inp[f"w_v{i}"], f).T
                                * np.float32(SV))
        out[f"wo{i}"] = np.ascontiguousarray(
            np.asarray(inp[f"w_o{i}"], f).T.astype(bf))
    g3 = np.asarray(inp["ln3_g"], f)
    wf1 = g3[:, None] * np.asarray(inp["w_ff1"], f).T      # [1024, 8192]
    # a-half bf16: [pm, k, (kd, col)] with d = 128kd + k
    out["wf1a"] = np.ascontiguousarray(
        wf1[:, :FF].reshape(8, P, 32, P).transpose(2, 1, 0, 3)
        .reshape(32, P, 1024).astype(bf))
    # gate-half fp8 DR: [pm, k, (j, i, col)] with d = 256j + 128i + k
    out["wf1g"] = np.ascontiguousarray(
        (wf1[:, FF:] * np.float32(SF))
        .reshape(4, 2, P, 32, P).transpose(3, 2, 0, 1, 4)
        .reshape(32, P, 1024).astype(f8))
    w2 = np.asarray(inp["w_ff2"], f).T * np.float32(SF)     # [4096, 1024]
    # ff-dim = 256jj + 128i + k -> [16, 128, 2, 1024]
    out["wf2"] = np.ascontiguousarray(
        w2.reshape(16, 2, P, D).transpose(0, 2, 1, 3)
        .reshape(16, P, 2 * D).astype(f8))
    return out


def prep_biases(inp):
    """Bias vectors pushed through the projections (all-zero in practice).
    Scaled to match the pre-scaled fp8 weights."""
    import ml_dtypes
    f = np.float32
    out = {}
    sc = np.float32(DH ** -0.5)
    for i in (1, 2):
        b = np.asarray(inp[f"ln{i}_b"], f)
        out[f"cq{i}"] = (np.asarray(inp[f"w_q{i}"], f) @ b * sc * SQ)[None, :]
        out[f"ck{i}"] = (np.asarray(inp[f"w_k{i}"], f) @ b * SK)[None, :]
        out[f"cv{i}"] = (np.asarray(inp[f"w_v{i}"], f) @ b * SV)[None, :]
        out[f"bo{i}"] = np.asarray(inp[f"b_o{i}"], f)[None, :]
    b3 = np.asarray(inp["ln3_b"], f)
    c1 = np.asarray(inp["w_ff1"], f) @ b3 + np.asarray(inp["b_ff1"], f)
    c1 = np.concatenate([c1[:FF], c1[FF:] * SF])
    out["c1"] = c1[None, :]
    out["bf2"] = (np.asarray(inp["b_ff2"], f) * SF)[None, :]
    return {k: np.ascontiguousarray(v.astype(ml_dtypes.bfloat16))
            for k, v in out.items()}


def any_bias(inp):
    keys = ["ln1_b", "ln2_b", "ln3_b", "b_o1", "b_o2", "b_ff1", "b_ff2"]
    return any(np.any(np.asarray(inp[k]) != 0) for k in keys)


# ======================================================================
# Host-side entry point: kernel(**inputs) -> full output [2, 2048, 1024]
# ======================================================================

_B, _N = 2, 2048
_NCORE = 8
_GROUP = 4
_TOK = _N // _GROUP

_cache = {}


def _get_nc(use_bias):
    key = ("nc", use_bias)
    if key not in _cache:
        _cache[key] = build(group=_GROUP, tok=_TOK, use_bias=use_bias)
    return _cache[key]


def kernel(**inputs):
    from concourse.bass_utils import run_bass_kernel_spmd

    inputs = {k: np.asarray(v) for k, v in inputs.items()}
    use_bias = any_bias(inputs)
    nc = _get_nc(use_bias)
    wdev = prep_weights(inputs)
    if use_bias:
        wdev.update(prep_biases(inputs))

    x = np.asarray(inputs["x"], np.float32)
    in_maps = []
    for core in range(_NCORE):
        b, p = core // _GROUP, core % _GROUP
        xs = np.ascontiguousarray(x[b, p * _TOK:(p + 1) * _TOK, :])
        remotes = [r for r in range(_GROUP) if r != p]
        rbo = np.zeros((P, _GROUP - 1), np.int32)
        for i, r in enumerate(remotes):
            rbo[:, i] = r * P + np.arange(P)
        in_maps.append({"x": xs, "rbo": rbo, **wdev})

    res = run_bass_kernel_spmd(nc, in_maps, list(range(_NCORE)))

    y = np.zeros((_B, _N, D), np.float32)
    for core in range(_NCORE):
        b, p = core // _GROUP, core % _GROUP
        y[b, p * _TOK:(p + 1) * _TOK, :] = res.results[core]["y"]
    return y
